# revision 38
# baseline (speedup 1.0000x reference)
"""Causal self-attention with RoPE on 8 Trainium2 NeuronCores.

Sharding: tensor-parallel over heads (4 groups of 4 heads) x data-parallel
over batch (2), one (batch, head-group) pair per core. Each core computes
its heads' QKV projection, RoPE, causal attention, and a row-slice of the
output projection; the host sums the 4 partial projections per batch.

Precision: all GEMMs run bf16 (fp8 operand noise of 3.6% fails the 2e-2
gate on every path except the probabilities) - except the softmax
probabilities, written once as fp8-e4m3 by the exp. That enables the
rowsum as an fp8 DoubleRow matmul (half cost) and P@V as a mixed
bf16-stationary x fp8-moving matmul (full rate, V stays clean). The
probabilities' quantization noise largely cancels between the P@V
numerator and the rowsum denominator. exp computes exp(s*scale - 2.5);
the bias keeps values under e4m3's 240 max and cancels in normalization.

RoPE exploits that attention is invariant to any per-head permutation of
the q/k feature dim: outputs are stored as [x1*c-x2*s | x1*s+x2*c] halves
(no interleaving), so the DVE chain is 4 batched ops per 128-token tile.

Phase 2 orders head-blocks group-major (g=0,3,2,1) and spreads the output
projection's 64 independent (c4, m) units as PE filler between attention
pairs, so the exp latency never leaves the PE idle.

Hardcoded problem shape: x (2,2048,2048), Wqkv (2048,6144), Wproj
(2048,2048), cos/sin (2048,64), 16 heads, head_dim 128.
"""

import sys

sys.path.insert(0, "/opt/trn_rl_repo")

import numpy as np
import ml_dtypes

import concourse.bass as bass
import concourse.tile as tile
from concourse import bacc, mybir
from concourse.bass_utils import run_bass_kernel_spmd

B, T, D, H = 2, 2048, 2048, 16
HD, HALF = 128, 64
TPC = 4          # heads per core
NT = T // 128    # 16 t-tiles
NK = D // 128    # 16 contraction chunks for the projections
NG = T // 512    # 4 q-groups per head
SCALE = float(1.0 / np.sqrt(HD))
EXP_BIAS = -2.5
FP32 = mybir.dt.float32
BF16 = mybir.dt.bfloat16
FP8 = mybir.dt.float8e4
NP_BF16 = ml_dtypes.bfloat16
NP_FP8 = ml_dtypes.float8_e4m3
DR = mybir.MatmulPerfMode.DoubleRow
EXP = mybir.ActivationFunctionType.Exp


def build_program():
    nc = bacc.Bacc("TRN2", target_bir_lowering=False, debug=False)

    xT = nc.dram_tensor("xT", [D, T], BF16, kind="ExternalInput").ap()
    wqk = nc.dram_tensor("wqk", [D, 2 * TPC * HD], BF16, kind="ExternalInput").ap()
    wv = nc.dram_tensor("wv", [D, TPC * HD], BF16, kind="ExternalInput").ap()
    wp = nc.dram_tensor("wp", [TPC * HD, D], BF16, kind="ExternalInput").ap()
    # cos/sin pre-tiled: [p, t, i] = table[128t + p, i]
    cosP = nc.dram_tensor("cosP", [128, NT, HALF], FP32, kind="ExternalInput").ap()
    sinP = nc.dram_tensor("sinP", [128, NT, HALF], FP32, kind="ExternalInput").ap()
    maskl = nc.dram_tensor("maskl", [128, 128], FP32, kind="ExternalInput").ap()
    ident = nc.dram_tensor("ident", [128, 128], BF16, kind="ExternalInput").ap()
    ones8 = nc.dram_tensor("ones8", [128, 2, 128], FP8, kind="ExternalInput").ap()
    outT = nc.dram_tensor("outT", [D, T], BF16, kind="ExternalOutput").ap()

    with tile.TileContext(nc) as tc:
        _kernel(tc, xT, wqk, wv, wp, cosP, sinP, maskl, ident, ones8, outT)
    nc.compile()
    return nc


def _kernel(tc, xT, wqk, wv, wp, cosP, sinP, maskl, ident, ones8, outT):
    nc = tc.nc
    NQK = 2 * TPC * HD  # 1024 qk output columns
    NV = TPC * HD       # 512 v output columns

    from contextlib import ExitStack

    with ExitStack() as top:
        # ---- persistent pools ----
        consts = top.enter_context(tc.tile_pool(name="consts", bufs=1))
        wp_pool = top.enter_context(tc.tile_pool(name="wp", bufs=TPC))
        qk_pool = top.enter_context(tc.tile_pool(name="qk", bufs=2))
        v_pool = top.enter_context(tc.tile_pool(name="v", bufs=NT))
        o_pool = top.enter_context(tc.tile_pool(name="o", bufs=TPC))

        # phase-2-only constants are DMA'd at phase-2 start so they don't
        # inflate the first projection matmul's DMA semaphore threshold
        l_tile = consts.tile([128, 128], FP32)
        ones_t = consts.tile([128, 2, 128], FP8)
        bias_t = consts.tile([128, 1], FP32)
        id_tile = consts.tile([128, 128], BF16)

        # QT/KT: (128 hd, head, T) single tiles so one DVE copy evicts all
        # four heads' transposes; V: per t-tile (128 t, 4*HD)
        QT = qk_pool.tile([128, TPC, T], BF16, name="QT")
        KT = qk_pool.tile([128, TPC, T], BF16, name="KT")
        V = [v_pool.tile([128, NV], BF16, tag="v", name=f"V{i}")
             for i in range(NT)]
        # attention output transposed: per head, (128 hd, T)
        OT = [o_pool.tile([128, T], BF16, tag="o", name=f"OT{i}")
              for i in range(TPC)]

        # ===== phase 1: fused q/k/v projection + rope + transpose ==========
        # Single pass over x: per 128-row t-tile, accumulate Q,K (one 2-bank
        # psum) and V over the 16 D-chunks, then rope+transpose Q/K and
        # evict V.
        with tc.tile_pool(name="wqk_cache", bufs=NK // 4) as wqk_pool, \
             tc.tile_pool(name="wv_cache", bufs=NK // 4) as wv_pool, \
             tc.tile_pool(name="x_stream", bufs=6) as x_pool, \
             tc.tile_pool(name="rope_tmp", bufs=2) as rope_pool, \
             tc.tile_pool(name="ro_sb", bufs=3) as ro_pool, \
             tc.tile_pool(name="cs", bufs=1) as cs_pool, \
             tc.tile_pool(name="psA", bufs=2, space="PSUM") as psA, \
             tc.tile_pool(name="psV", bufs=2, space="PSUM") as psVp, \
             tc.tile_pool(name="psT", bufs=2, space="PSUM") as psT:

            cos_t = cs_pool.tile([128, NT * HALF], FP32)
            sin_t = cs_pool.tile([128, NT * HALF], FP32)

            # weight-cache tiles are DMA'd lazily inside the t=0 loop so the
            # first matmul's DMA semaphore wait covers only what it needs
            WQK = []
            WV = []

            def emit_transposes(pend):
                # transposes of tile t-1, emitted under tile t's matmuls so
                # the PE never waits on the serial DVE rope chain
                for ro, g, pst, hh in pend:
                    nc.tensor.transpose(
                        pst[:, hh * 128:(hh + 1) * 128], ro[:, g], id_tile)

            pend_tr = []
            pend_ev = []
            WP = []
            XW = [None] * NK
            for t in range(NT):
                psQK = psA.tile([128, 1024], FP32, tag="psA")
                psV = psVp.tile([128, 512], FP32, tag="psV")
                tg0 = (t // 4) * 512   # t-group column base
                tc0 = (t % 4) * 128    # this tile's offset within the group
                for k in range(NK):
                    kj, kk = k // 4, k % 4
                    if t == 0 and kk == 0:
                        # 4-chunk merged weight DMAs: 4x fewer Sync issues
                        # (the ~0.6us per-DMA descriptor cost made t=0
                        # issue-bound) and smaller conservative semaphore
                        # thresholds for the early matmuls. The very first
                        # group is halved, with the second half (and the
                        # transpose identity, first used at t=1) emitted
                        # after the k=0 matmuls so matmul-0's conservative
                        # threshold covers only ~0.75MB of transfers.
                        w = wqk_pool.tile([128, 4, NQK], BF16, tag="wqk")
                        if kj == 0:
                            nc.sync.dma_start(
                                out=w[:, 0:2, :],
                                in_=wqk[0:256, :].rearrange(
                                    "(kk p) c -> p kk c", kk=2))
                        else:
                            nc.sync.dma_start(
                                out=w,
                                in_=wqk[kj * 512:(kj + 1) * 512, :].rearrange(
                                    "(kk p) c -> p kk c", kk=4))
                        WQK.append(w)
                    if t == 0 and k == 1:
                        nc.sync.dma_start(out=id_tile, in_=ident)
                        nc.sync.dma_start(
                            out=WQK[0][:, 2:4, :],
                            in_=wqk[256:512, :].rearrange(
                                "(kk p) c -> p kk c", kk=2))
                        nc.sync.dma_start(
                            out=XW[0][:, 2:4, :],
                            in_=xT[256:512, tg0:tg0 + 512].rearrange(
                                "(kk p) t -> p kk t", kk=2))
                    if t % 4 == 0 and kk == 0:
                        # x streams in merged (4 d-chunk, 512 t) tiles
                        xw = x_pool.tile([128, 4, 512], BF16, tag="x")
                        if t == 0 and kj == 0:
                            nc.sync.dma_start(
                                out=xw[:, 0:2, :],
                                in_=xT[0:256, tg0:tg0 + 512].rearrange(
                                    "(kk p) t -> p kk t", kk=2))
                        else:
                            nc.sync.dma_start(
                                out=xw,
                                in_=xT[kj * 512:(kj + 1) * 512,
                                       tg0:tg0 + 512].rearrange(
                                    "(kk p) t -> p kk t", kk=4))
                        XW[kj] = xw
                        if t == 0 and k == 0:
                            # cos/sin ride the GpSimd DMA queue so they
                            # don't delay the weight stream on Sync
                            nc.gpsimd.dma_start(out=cos_t, in_=cosP)
                            nc.gpsimd.dma_start(out=sin_t, in_=sinP)
                    xt = XW[kj][:, kk, tc0:tc0 + 128]
                    nc.tensor.matmul(psQK[:, 0:512], xt, WQK[kj][:, kk, 0:512],
                                     start=(k == 0), stop=(k == NK - 1))
                    nc.tensor.matmul(psQK[:, 512:1024], xt, WQK[kj][:, kk, 512:1024],
                                     start=(k == 0), stop=(k == NK - 1))
                    if t == 0 and kk == 0:
                        # wv load emitted after the q/k matmuls that don't
                        # need it: keeps it out of their DMA sem threshold
                        w = wv_pool.tile([128, 4, NV], BF16, tag="wv")
                        nc.sync.dma_start(
                            out=w,
                            in_=wv[kj * 512:(kj + 1) * 512, :].rearrange(
                                "(kk p) c -> p kk c", kk=4))
                        WV.append(w)
                    nc.tensor.matmul(psV, xt, WV[kj][:, kk, :],
                                     start=(k == 0), stop=(k == NK - 1))
                    if k in (3, 5, 7, 9) and pend_tr:
                        # prior tile's rope outputs are ready by now; slot its
                        # transposes pairwise between this tile's matmuls
                        emit_transposes(pend_tr[:2])
                        pend_tr = pend_tr[2:]
                    if k == 11 and pend_ev:
                        # evict prior tile's transposes (DVE, packed bf16)
                        for pst, dst, tcol in pend_ev:
                            nc.vector.tensor_copy(
                                out=dst[:, :, tcol:tcol + 128],
                                in_=pst.rearrange("p (h c) -> p h c", h=TPC))
                        pend_ev = []
                # evict V on ACT: DVE is busy with rope in this phase
                nc.scalar.copy(out=V[t], in_=psV)

                # ---- rope, batched over {q,k} x 4 heads (g = qk*4+hh) ----
                # psQK layout: [qk, head, half, i]; out halves stay split
                # ([x1c-x2s | x1s+x2c]) - attention is invariant to the
                # per-head feature permutation, V/outproj are untouched.
                src = psQK.rearrange("p (g half i) -> p g half i",
                                     g=2 * TPC, half=2)
                c_b = cos_t[:, t * HALF:(t + 1) * HALF].unsqueeze(1).unsqueeze(1) \
                    .broadcast_to((128, 2 * TPC, 2, HALF))
                s_b = sin_t[:, t * HALF:(t + 1) * HALF].unsqueeze(1).unsqueeze(1) \
                    .broadcast_to((128, 2 * TPC, 2, HALF))
                t_a = rope_pool.tile([128, 2 * TPC, 2, HALF], BF16, tag="ta")
                t_b = rope_pool.tile([128, 2 * TPC, 2, HALF], BF16, tag="tb")
                nc.vector.tensor_mul(t_a, src, c_b)
                nc.vector.tensor_mul(t_b, src, s_b)
                ro = ro_pool.tile([128, 2 * TPC, 2, HALF], BF16, tag="ro")
                nc.vector.tensor_sub(ro[:, :, 0], t_a[:, :, 0], t_b[:, :, 1])
                nc.vector.tensor_add(ro[:, :, 1], t_b[:, :, 0], t_a[:, :, 1])
                pstQ = psT.tile([128, 512], BF16, tag="psT")
                pstK = psT.tile([128, 512], BF16, tag="psT")
                for hh in range(TPC):
                    pend_tr.append((ro, hh, pstQ, hh))
                for hh in range(TPC):
                    pend_tr.append((ro, TPC + hh, pstK, hh))
                pend_ev = [(pstQ, QT, t * 128), (pstK, KT, t * 128)]
            emit_transposes(pend_tr)
            for pst, dst, tcol in pend_ev:
                nc.vector.tensor_copy(
                    out=dst[:, :, tcol:tcol + 128],
                    in_=pst.rearrange("p (h c) -> p h c", h=TPC))

        # ===== phase 2: causal attention + interleaved output proj =========
        # Score chunk pairs land in one 2-bank psum tile; one exp covers the
        # pair's union window (ragged holes are computed then -1e30'd so
        # they exp to zero), producing fp8 probabilities. P@V runs as two
        # mixed bf16xfp8 matmuls per pair; the rowsum as one fp8 DoubleRow.
        # Output-projection units are spread between pairs as PE filler.
        with tc.tile_pool(name="p_sb", bufs=4) as p_pool, \
             tc.tile_pool(name="rs_sb", bufs=4) as rs_pool, \
             tc.tile_pool(name="out_evict", bufs=6) as out_pool, \
             tc.tile_pool(name="psS", bufs=2, space="PSUM") as psS, \
             tc.tile_pool(name="psO", bufs=4, space="PSUM") as psO:
            # prefetch phase-2 constants + Wproj now (not at kernel start)
            nc.sync.dma_start(out=l_tile, in_=maskl)
            nc.sync.dma_start(out=ones_t, in_=ones8)
            nc.gpsimd.memset(bias_t, EXP_BIAS)
            for hh in range(TPC):
                w = wp_pool.tile([128, D], BF16, tag="wp", name=f"WP{hh}")
                nc.sync.dma_start(out=w, in_=wp[hh * 128:(hh + 1) * 128, :])
                WP.append(w)
            filler = []      # ready outproj (c4, m) units
            n_evict = [0]

            def emit_outproj_unit(c4, m):
                ps = psO.tile([128, 512], FP32, tag="psO")
                for hh2 in range(TPC):
                    nc.tensor.matmul(
                        ps,
                        WP[hh2][:, m * 128:(m + 1) * 128],
                        OT[hh2][:, c4 * 512:(c4 + 1) * 512],
                        start=(hh2 == 0), stop=(hh2 == TPC - 1),
                    )
                ob = out_pool.tile([128, 512], BF16, tag="ob")
                # alternate evict engine: DVE carries most, ACT takes every
                # third (ACT has headroom over the exp stream); the out DMA
                # issue rotates across queues so the drain never serializes
                # on the Sync engine's ~0.6us per-DMA descriptor cost
                if n_evict[0] % 3 == 2:
                    nc.scalar.copy(out=ob, in_=ps)
                else:
                    nc.vector.tensor_copy(out=ob, in_=ps)
                n_evict[0] += 1
                nc.sync.dma_start(
                    out=outT[m * 128:(m + 1) * 128, c4 * 512:(c4 + 1) * 512],
                    in_=ob,
                )

            def emit_filler(k):
                for _ in range(k):
                    if filler:
                        emit_outproj_unit(*filler.pop(0))

            # group-major block order: finishing all heads of a q-group
            # releases that group's 16 outproj units as filler for the rest.
            # g=0 first: the smallest group minimizes the filler-less prefix.
            blocks = [(g, hh) for g in (0, 3, 2, 1) for hh in range(TPC)]
            per_pair = {0: 0, 3: 1, 2: 1, 1: 1}
            store = {}

            def emit_sp(bi, p):
                g, hh = blocks[bi]
                qcol0 = g * 512
                ps = psS.tile([128, 1024], FP32, tag="psS")
                offA = max(0, 2 * p - 4 * g) * 128
                offB = max(0, 2 * p + 1 - 4 * g) * 128
                for h, kj in ((0, 2 * p), (1, 2 * p + 1)):
                    # both chunks computed over the pair's union window
                    # [offA, 512) so one exp / one DoubleRow covers both
                    nc.tensor.matmul(
                        ps[:, 512 * h + offA:512 * h + 512],
                        KT[:, hh, kj * 128:(kj + 1) * 128],
                        QT[:, hh, qcol0 + offA:qcol0 + 512],
                        start=True, stop=True,
                    )
                    # causal mask emitted WITH the scores: resolved well
                    # before the consuming exp
                    sd = kj - 4 * g
                    if 0 <= sd <= 3:
                        dcol = 512 * h + sd * 128
                        nc.vector.tensor_sub(
                            ps[:, dcol:dcol + 128],
                            ps[:, dcol:dcol + 128],
                            l_tile,
                        )
                if offB > offA:
                    # chunk B's pre-diagonal block is fully masked: overwrite
                    # with -1e30 (dep-ordered after its matmul) so it exps to
                    # zero and the pair stays valid over [offA, 512)
                    nc.vector.memset(ps[:, 512 + offA:512 + offB], -1e30)
                store[(bi, p)] = (ps, offA)

            emit_sp(0, 0)
            pending_rescale = []

            def flush_rescale():
                for po_, rs_, hh_, qc_ in pending_rescale:
                    rrep = rs_pool.tile([128, 512], FP32, tag="rrep")
                    nc.vector.reciprocal_approx_fast(rrep, rs_)
                    nc.vector.tensor_mul(OT[hh_][:, qc_:qc_ + 512], po_, rrep)
                    if hh_ == TPC - 1:
                        filler.extend((qc_ // 512, m) for m in range(NK))
                pending_rescale.clear()

            for bi, (g, hh) in enumerate(blocks):
                qcol0 = g * 512
                nchunks = 4 * g + 4
                npairs = nchunks // 2

                po = psO.tile([128, 512], FP32, tag="psO")
                rs = psO.tile([128, 512], FP32, tag="psO")
                for p in range(npairs):
                    if p + 1 < npairs:
                        emit_sp(bi, p + 1)
                    elif bi + 1 < len(blocks):
                        # last pair: pre-emit the NEXT block's first scores
                        # pair so its exp starts before this block drains
                        emit_sp(bi + 1, 0)
                    ps, offA = store.pop((bi, p))
                    pt = p_pool.tile([128, 1024], FP8, tag="p")
                    ps_v = ps.rearrange("p (two q) -> p two q", two=2)[:, :, offA:512]
                    pt_v = pt.rearrange("p (two q) -> p two q", two=2)[:, :, offA:512]
                    if p == 0:
                        # split the block's first exp into halves: P@V on
                        # chunk A starts half an exp earlier (block starts
                        # are where the exp latency is exposed; offA==0 here)
                        for h2 in (0, 1):
                            nc.scalar.activation(
                                out=pt[:, 512 * h2:512 * h2 + 512],
                                in_=ps[:, 512 * h2:512 * h2 + 512],
                                func=EXP, scale=SCALE, bias=bias_t)
                    else:
                        nc.scalar.activation(out=pt_v, in_=ps_v, func=EXP,
                                             scale=SCALE, bias=bias_t)
                    # rowsum over k (ones DoubleRow matmul, psum-accum)
                    nc.tensor.matmul(rs[:, offA:512], ones_t, pt_v,
                                     start=(p == 0), stop=(p == npairs - 1),
                                     perf_mode=DR)
                    # PV accumulate: two mixed bf16-stat x fp8-mov matmuls;
                    # chunk B's zeroed hole contributes nothing over the
                    # union window
                    for h, kj in ((0, 2 * p), (1, 2 * p + 1)):
                        nc.tensor.matmul(
                            po[:, offA:512],
                            V[kj][:, hh * HD:(hh + 1) * HD],
                            pt[:, 512 * h + offA:512 * h + 512],
                            start=(p == 0 and h == 0),
                            stop=(p == npairs - 1 and h == 1),
                        )
                    if p == 0 and pending_rescale:
                        # deferred rescale: every block's recip/mul chain is
                        # emitted after the NEXT block's first pair, so that
                        # pair's mask-subs enter the in-order DVE queue ahead
                        # of it (the chain otherwise stalls the next scores)
                        flush_rescale()
                    emit_filler(per_pair[g])
                pending_rescale.append((po, rs, hh, qcol0))

            flush_rescale()
            # ========= phase 3: drain remaining outproj units ==========
            while filler:
                emit_outproj_unit(*filler.pop(0))


_PROGRAM = None


def _get_program():
    global _PROGRAM
    if _PROGRAM is None:
        _PROGRAM = build_program()
    return _PROGRAM


def _make_in_maps(x, cos, sin, Wqkv, Wproj):
    maskl = (np.tril(np.ones((128, 128), np.float32), -1) * 1e30).astype(np.float32)
    ident = np.eye(128, dtype=np.float32).astype(NP_BF16)
    ones8 = np.ones((128, 2, 128), dtype=np.float32).astype(NP_FP8)
    cosP = np.ascontiguousarray(
        np.asarray(cos, np.float32).reshape(NT, 128, HALF).transpose(1, 0, 2))
    sinP = np.ascontiguousarray(
        np.asarray(sin, np.float32).reshape(NT, 128, HALF).transpose(1, 0, 2))
    in_maps = []
    for c in range(8):
        b, hg = c // 4, c % 4
        h0 = hg * TPC
        in_maps.append({
            "xT": np.ascontiguousarray(x[b].T).astype(NP_BF16),
            "wqk": np.ascontiguousarray(np.concatenate(
                [Wqkv[:, h0 * HD:(h0 + TPC) * HD],
                 Wqkv[:, D + h0 * HD:D + (h0 + TPC) * HD]],
                axis=1)).astype(NP_BF16),
            "wv": np.ascontiguousarray(
                Wqkv[:, 2 * D + h0 * HD:2 * D + (h0 + TPC) * HD]).astype(NP_BF16),
            "wp": np.ascontiguousarray(
                Wproj[h0 * HD:(h0 + TPC) * HD, :]).astype(NP_BF16),
            "cosP": cosP,
            "sinP": sinP,
            "maskl": maskl,
            "ident": ident,
            "ones8": ones8,
        })
    return in_maps


def _combine(results):
    outs = []
    for b in range(2):
        acc = results[4 * b]["outT"].astype(np.float32)
        for hg in range(1, 4):
            acc = acc + results[4 * b + hg]["outT"].astype(np.float32)
        outs.append(acc.T)
    return np.ascontiguousarray(np.stack(outs))


def kernel(x, cos, sin, Wqkv, Wproj):
    nc = _get_program()
    in_maps = _make_in_maps(np.asarray(x, np.float32), cos, sin,
                            np.asarray(Wqkv, np.float32),
                            np.asarray(Wproj, np.float32))
    res = run_bass_kernel_spmd(nc, in_maps, list(range(8)))
    return _combine(res.results)


def _install_ntff_shim():
    """Provide the antenv.axon_hooks registry this container lacks, wired to
    the ctypes NTFF hook from trn_agent_boot, so trace=True works."""
    import types

    if "antenv.axon_hooks" in sys.modules:
        return
    hook = None
    try:
        from trn_agent_boot.trn_boot import _ntff_profile_via_ctypes
        hook = _ntff_profile_via_ctypes("/opt/axon/libaxon_pjrt.so")
    except Exception as e:
        print("ntff shim unavailable:", e)
    mod = types.ModuleType("antenv.axon_hooks")
    mod._hook = hook
    mod.get_axon_ntff_profile_hook = lambda: mod._hook
    mod.set_axon_ntff_profile_hook = lambda h: setattr(mod, "_hook", h)
    sys.modules["antenv.axon_hooks"] = mod
    # keep artifacts local; the bucket upload path isn't available here
    import concourse.bass_utils as bu
    bu.upload_artifacts = lambda tmpdir: tmpdir


def kernel_profiled(x, cos, sin, Wqkv, Wproj, trace_cores=None, tmpdir=None):
    nc = _get_program()
    _install_ntff_shim()
    in_maps = _make_in_maps(np.asarray(x, np.float32), cos, sin,
                            np.asarray(Wqkv, np.float32),
                            np.asarray(Wproj, np.float32))
    res = run_bass_kernel_spmd(nc, in_maps, list(range(8)), trace=True,
                               trace_cores=trace_cores, tmpdir=tmpdir)
    return _combine(res.results), res


# revision 40
# speedup vs baseline: 1.0406x; 1.0406x over previous
"""Causal self-attention with RoPE on 8 Trainium2 NeuronCores.

Sharding: tensor-parallel over heads (4 groups of 4 heads) x data-parallel
over batch (2), one (batch, head-group) pair per core. Each core computes
its heads' QKV projection, RoPE, causal attention, and a row-slice of the
output projection; the host sums the 4 partial projections per batch.

Precision: all GEMMs run bf16 (fp8 operand noise of 3.6% fails the 2e-2
gate on every path except the probabilities) - except the softmax
probabilities, written once as fp8-e4m3 by the exp. That enables the
rowsum as an fp8 DoubleRow matmul (half cost) and P@V as a mixed
bf16-stationary x fp8-moving matmul (full rate, V stays clean). The
probabilities' quantization noise largely cancels between the P@V
numerator and the rowsum denominator. exp computes exp(s*scale - 2.5);
the bias keeps values under e4m3's 240 max and cancels in normalization.

RoPE exploits that attention is invariant to any per-head permutation of
the q/k feature dim: outputs are stored as [x1*c-x2*s | x1*s+x2*c] halves
(no interleaving), so the DVE chain is 4 batched ops per 128-token tile.

Phase 2 orders head-blocks group-major (g=0,3,2,1) and spreads the output
projection's 64 independent (c4, m) units as PE filler between attention
pairs, so the exp latency never leaves the PE idle.

Hardcoded problem shape: x (2,2048,2048), Wqkv (2048,6144), Wproj
(2048,2048), cos/sin (2048,64), 16 heads, head_dim 128.
"""

import sys

sys.path.insert(0, "/opt/trn_rl_repo")

import numpy as np
import ml_dtypes

import concourse.bass as bass
import concourse.tile as tile
from concourse import bacc, mybir
from concourse.bass_utils import run_bass_kernel_spmd

B, T, D, H = 2, 2048, 2048, 16
HD, HALF = 128, 64
TPC = 4          # heads per core
NT = T // 128    # 16 t-tiles
NK = D // 128    # 16 contraction chunks for the projections
NG = T // 512    # 4 q-groups per head
SCALE = float(1.0 / np.sqrt(HD))
EXP_BIAS = -2.5
FP32 = mybir.dt.float32
BF16 = mybir.dt.bfloat16
FP8 = mybir.dt.float8e4
NP_BF16 = ml_dtypes.bfloat16
NP_FP8 = ml_dtypes.float8_e4m3
DR = mybir.MatmulPerfMode.DoubleRow
EXP = mybir.ActivationFunctionType.Exp


def build_program():
    nc = bacc.Bacc("TRN2", target_bir_lowering=False, debug=False)

    xT = nc.dram_tensor("xT", [D, T], BF16, kind="ExternalInput").ap()
    wqk = nc.dram_tensor("wqk", [D, 2 * TPC * HD], BF16, kind="ExternalInput").ap()
    wv = nc.dram_tensor("wv", [D, TPC * HD], BF16, kind="ExternalInput").ap()
    wp = nc.dram_tensor("wp", [TPC * HD, D], BF16, kind="ExternalInput").ap()
    # cos/sin pre-tiled: [p, t, i] = table[128t + p, i]
    cosP = nc.dram_tensor("cosP", [128, NT, HALF], FP32, kind="ExternalInput").ap()
    sinP = nc.dram_tensor("sinP", [128, NT, HALF], FP32, kind="ExternalInput").ap()
    maskl = nc.dram_tensor("maskl", [128, 128], FP32, kind="ExternalInput").ap()
    ident = nc.dram_tensor("ident", [128, 128], BF16, kind="ExternalInput").ap()
    ones8 = nc.dram_tensor("ones8", [128, 2, 128], FP8, kind="ExternalInput").ap()
    outT = nc.dram_tensor("outT", [D, T], BF16, kind="ExternalOutput").ap()

    with tile.TileContext(nc) as tc:
        _kernel(tc, xT, wqk, wv, wp, cosP, sinP, maskl, ident, ones8, outT)
    nc.compile()
    return nc


def _kernel(tc, xT, wqk, wv, wp, cosP, sinP, maskl, ident, ones8, outT):
    nc = tc.nc
    NQK = 2 * TPC * HD  # 1024 qk output columns
    NV = TPC * HD       # 512 v output columns

    from contextlib import ExitStack

    with ExitStack() as top:
        # ---- persistent pools ----
        consts = top.enter_context(tc.tile_pool(name="consts", bufs=1))
        wp_pool = top.enter_context(tc.tile_pool(name="wp", bufs=TPC))
        qk_pool = top.enter_context(tc.tile_pool(name="qk", bufs=2))
        v_pool = top.enter_context(tc.tile_pool(name="v", bufs=NT))
        o_pool = top.enter_context(tc.tile_pool(name="o", bufs=TPC))

        # phase-2-only constants are DMA'd at phase-2 start so they don't
        # inflate the first projection matmul's DMA semaphore threshold
        l_tile = consts.tile([128, 128], FP32)
        ones_t = consts.tile([128, 2, 128], FP8)
        bias_t = consts.tile([128, 1], FP32)
        id_tile = consts.tile([128, 128], BF16)

        # QT/KT: (128 hd, head, T) single tiles so one DVE copy evicts all
        # four heads' transposes; V: per t-tile (128 t, 4*HD)
        QT = qk_pool.tile([128, TPC, T], BF16, name="QT")
        KT = qk_pool.tile([128, TPC, T], BF16, name="KT")
        V = [v_pool.tile([128, NV], BF16, tag="v", name=f"V{i}")
             for i in range(NT)]
        # attention output transposed: per head, (128 hd, T)
        OT = [o_pool.tile([128, T], BF16, tag="o", name=f"OT{i}")
              for i in range(TPC)]

        # ===== phase 1: fused q/k/v projection + rope + transpose ==========
        # Single pass over x: per 128-row t-tile, accumulate Q,K (one 2-bank
        # psum) and V over the 16 D-chunks, then rope+transpose Q/K and
        # evict V.
        with tc.tile_pool(name="wqk_cache", bufs=NK // 4) as wqk_pool, \
             tc.tile_pool(name="wv_cache", bufs=NK // 4) as wv_pool, \
             tc.tile_pool(name="x_stream", bufs=6) as x_pool, \
             tc.tile_pool(name="rope_tmp", bufs=2) as rope_pool, \
             tc.tile_pool(name="ro_sb", bufs=3) as ro_pool, \
             tc.tile_pool(name="cs", bufs=1) as cs_pool, \
             tc.tile_pool(name="psA", bufs=2, space="PSUM") as psA, \
             tc.tile_pool(name="psV", bufs=2, space="PSUM") as psVp, \
             tc.tile_pool(name="psT", bufs=2, space="PSUM") as psT:

            cos_t = cs_pool.tile([128, NT * HALF], FP32)
            sin_t = cs_pool.tile([128, NT * HALF], FP32)

            # weight-cache tiles are DMA'd lazily inside the t=0 loop so the
            # first matmul's DMA semaphore wait covers only what it needs
            WQK = []
            WV = []

            def emit_transposes(pend):
                # transposes of tile t-1, emitted under tile t's matmuls so
                # the PE never waits on the serial DVE rope chain
                for ro, g, pst, hh in pend:
                    nc.tensor.transpose(
                        pst[:, hh * 128:(hh + 1) * 128], ro[:, g], id_tile)

            pend_tr = []
            pend_ev = []
            WP = []
            XW = [None] * NK
            for t in range(NT):
                psQK = psA.tile([128, 1024], FP32, tag="psA")
                psV = psVp.tile([128, 512], FP32, tag="psV")
                tg0 = (t // 4) * 512   # t-group column base
                tc0 = (t % 4) * 128    # this tile's offset within the group
                for k in range(NK):
                    kj, kk = k // 4, k % 4
                    if t == 0 and kk == 0:
                        # 4-chunk merged weight DMAs: 4x fewer Sync issues
                        # (the ~0.6us per-DMA descriptor cost made t=0
                        # issue-bound) and smaller conservative semaphore
                        # thresholds for the early matmuls. The very first
                        # group is halved, with the second half (and the
                        # transpose identity, first used at t=1) emitted
                        # after the k=0 matmuls so matmul-0's conservative
                        # threshold covers only ~0.75MB of transfers.
                        w = wqk_pool.tile([128, 4, NQK], BF16, tag="wqk")
                        if kj == 0:
                            nc.sync.dma_start(
                                out=w[:, 0:2, :],
                                in_=wqk[0:256, :].rearrange(
                                    "(kk p) c -> p kk c", kk=2))
                        else:
                            nc.sync.dma_start(
                                out=w,
                                in_=wqk[kj * 512:(kj + 1) * 512, :].rearrange(
                                    "(kk p) c -> p kk c", kk=4))
                        WQK.append(w)
                    if t == 0 and k == 1:
                        nc.sync.dma_start(out=id_tile, in_=ident)
                        nc.sync.dma_start(
                            out=WQK[0][:, 2:4, :],
                            in_=wqk[256:512, :].rearrange(
                                "(kk p) c -> p kk c", kk=2))
                        nc.sync.dma_start(
                            out=XW[0][:, 2:4, :],
                            in_=xT[256:512, tg0:tg0 + 512].rearrange(
                                "(kk p) t -> p kk t", kk=2))
                    if t % 4 == 0 and kk == 0:
                        # x streams in merged (4 d-chunk, 512 t) tiles
                        xw = x_pool.tile([128, 4, 512], BF16, tag="x")
                        if t == 0 and kj == 0:
                            nc.sync.dma_start(
                                out=xw[:, 0:2, :],
                                in_=xT[0:256, tg0:tg0 + 512].rearrange(
                                    "(kk p) t -> p kk t", kk=2))
                        else:
                            nc.sync.dma_start(
                                out=xw,
                                in_=xT[kj * 512:(kj + 1) * 512,
                                       tg0:tg0 + 512].rearrange(
                                    "(kk p) t -> p kk t", kk=4))
                        XW[kj] = xw
                        if t == 0 and k == 0:
                            # cos/sin ride the GpSimd DMA queue so they
                            # don't delay the weight stream on Sync
                            nc.gpsimd.dma_start(out=cos_t, in_=cosP)
                            nc.gpsimd.dma_start(out=sin_t, in_=sinP)
                    xt = XW[kj][:, kk, tc0:tc0 + 128]
                    nc.tensor.matmul(psQK[:, 0:512], xt, WQK[kj][:, kk, 0:512],
                                     start=(k == 0), stop=(k == NK - 1))
                    nc.tensor.matmul(psQK[:, 512:1024], xt, WQK[kj][:, kk, 512:1024],
                                     start=(k == 0), stop=(k == NK - 1))
                    if t == 0 and kk == 0:
                        # wv load emitted after the q/k matmuls that don't
                        # need it: keeps it out of their DMA sem threshold
                        w = wv_pool.tile([128, 4, NV], BF16, tag="wv")
                        nc.sync.dma_start(
                            out=w,
                            in_=wv[kj * 512:(kj + 1) * 512, :].rearrange(
                                "(kk p) c -> p kk c", kk=4))
                        WV.append(w)
                    nc.tensor.matmul(psV, xt, WV[kj][:, kk, :],
                                     start=(k == 0), stop=(k == NK - 1))
                    if k in (3, 5, 7, 9) and pend_tr:
                        # prior tile's rope outputs are ready by now; slot its
                        # transposes pairwise between this tile's matmuls
                        emit_transposes(pend_tr[:2])
                        pend_tr = pend_tr[2:]
                    if k == 11 and pend_ev:
                        # evict prior tile's transposes (DVE, packed bf16)
                        for pst, dst, tcol in pend_ev:
                            nc.vector.tensor_copy(
                                out=dst[:, :, tcol:tcol + 128],
                                in_=pst.rearrange("p (h c) -> p h c", h=TPC))
                        pend_ev = []
                # evict V on ACT: DVE is busy with rope in this phase
                nc.scalar.copy(out=V[t], in_=psV)

                # ---- rope, batched over {q,k} x 4 heads (g = qk*4+hh) ----
                # psQK layout: [qk, head, half, i]; out halves stay split
                # ([x1c-x2s | x1s+x2c]) - attention is invariant to the
                # per-head feature permutation, V/outproj are untouched.
                src = psQK.rearrange("p (g half i) -> p g half i",
                                     g=2 * TPC, half=2)
                c_b = cos_t[:, t * HALF:(t + 1) * HALF].unsqueeze(1).unsqueeze(1) \
                    .broadcast_to((128, 2 * TPC, 2, HALF))
                s_b = sin_t[:, t * HALF:(t + 1) * HALF].unsqueeze(1).unsqueeze(1) \
                    .broadcast_to((128, 2 * TPC, 2, HALF))
                t_a = rope_pool.tile([128, 2 * TPC, 2, HALF], BF16, tag="ta")
                t_b = rope_pool.tile([128, 2 * TPC, 2, HALF], BF16, tag="tb")
                nc.vector.tensor_mul(t_a, src, c_b)
                nc.vector.tensor_mul(t_b, src, s_b)
                ro = ro_pool.tile([128, 2 * TPC, 2, HALF], BF16, tag="ro")
                nc.vector.tensor_sub(ro[:, :, 0], t_a[:, :, 0], t_b[:, :, 1])
                nc.vector.tensor_add(ro[:, :, 1], t_b[:, :, 0], t_a[:, :, 1])
                pstQ = psT.tile([128, 512], BF16, tag="psT")
                pstK = psT.tile([128, 512], BF16, tag="psT")
                for hh in range(TPC):
                    pend_tr.append((ro, hh, pstQ, hh))
                for hh in range(TPC):
                    pend_tr.append((ro, TPC + hh, pstK, hh))
                pend_ev = [(pstQ, QT, t * 128), (pstK, KT, t * 128)]
            emit_transposes(pend_tr)
            for pst, dst, tcol in pend_ev:
                nc.vector.tensor_copy(
                    out=dst[:, :, tcol:tcol + 128],
                    in_=pst.rearrange("p (h c) -> p h c", h=TPC))

        # ===== phase 2: causal attention + interleaved output proj =========
        # Score chunk pairs land in one 2-bank psum tile; one exp covers the
        # pair's union window (ragged holes are computed then -1e30'd so
        # they exp to zero), producing fp8 probabilities. P@V runs as two
        # mixed bf16xfp8 matmuls per pair; the rowsum as one fp8 DoubleRow.
        # Output-projection units are spread between pairs as PE filler.
        with tc.tile_pool(name="p_sb", bufs=4) as p_pool, \
             tc.tile_pool(name="rs_sb", bufs=4) as rs_pool, \
             tc.tile_pool(name="out_evict", bufs=6) as out_pool, \
             tc.tile_pool(name="psS", bufs=2, space="PSUM") as psS, \
             tc.tile_pool(name="psO", bufs=4, space="PSUM") as psO:
            # prefetch phase-2 constants + Wproj now (not at kernel start)
            nc.sync.dma_start(out=l_tile, in_=maskl)
            nc.sync.dma_start(out=ones_t, in_=ones8)
            nc.gpsimd.memset(bias_t, EXP_BIAS)
            for hh in range(TPC):
                w = wp_pool.tile([128, D], BF16, tag="wp", name=f"WP{hh}")
                nc.sync.dma_start(out=w, in_=wp[hh * 128:(hh + 1) * 128, :])
                WP.append(w)
            filler = []      # ready outproj (c4, m) units
            n_evict = [0]

            def emit_outproj_unit(c4, m):
                ps = psO.tile([128, 512], FP32, tag="psO")
                for hh2 in range(TPC):
                    nc.tensor.matmul(
                        ps,
                        WP[hh2][:, m * 128:(m + 1) * 128],
                        OT[hh2][:, c4 * 512:(c4 + 1) * 512],
                        start=(hh2 == 0), stop=(hh2 == TPC - 1),
                    )
                ob = out_pool.tile([128, 512], BF16, tag="ob")
                # alternate evict engine: DVE carries most, ACT takes every
                # third (ACT has headroom over the exp stream); the out DMA
                # issue rotates across queues so the drain never serializes
                # on the Sync engine's ~0.6us per-DMA descriptor cost
                if n_evict[0] % 3 == 2:
                    nc.scalar.copy(out=ob, in_=ps)
                else:
                    nc.vector.tensor_copy(out=ob, in_=ps)
                n_evict[0] += 1
                nc.sync.dma_start(
                    out=outT[m * 128:(m + 1) * 128, c4 * 512:(c4 + 1) * 512],
                    in_=ob,
                )

            def emit_filler(k):
                for _ in range(k):
                    if filler:
                        emit_outproj_unit(*filler.pop(0))

            # group-major block order: finishing all heads of a q-group
            # releases that group's 16 outproj units as filler for the rest.
            # g=0 first: the smallest group minimizes the filler-less prefix.
            blocks = [(g, hh) for g in (0, 3, 2, 1) for hh in range(TPC)]
            per_pair = {0: 0, 3: 1, 2: 1, 1: 1}
            store = {}

            def emit_sp(bi, p):
                g, hh = blocks[bi]
                qcol0 = g * 512
                ps = psS.tile([128, 1024], FP32, tag="psS")
                offA = max(0, 2 * p - 4 * g) * 128
                offB = max(0, 2 * p + 1 - 4 * g) * 128
                for h, kj in ((0, 2 * p), (1, 2 * p + 1)):
                    # both chunks computed over the pair's union window
                    # [offA, 512) so one exp / one DoubleRow covers both
                    nc.tensor.matmul(
                        ps[:, 512 * h + offA:512 * h + 512],
                        KT[:, hh, kj * 128:(kj + 1) * 128],
                        QT[:, hh, qcol0 + offA:qcol0 + 512],
                        start=True, stop=True,
                    )
                    # causal mask emitted WITH the scores: resolved well
                    # before the consuming exp
                    sd = kj - 4 * g
                    if 0 <= sd <= 3:
                        dcol = 512 * h + sd * 128
                        nc.vector.tensor_sub(
                            ps[:, dcol:dcol + 128],
                            ps[:, dcol:dcol + 128],
                            l_tile,
                        )
                if offB > offA:
                    # chunk B's pre-diagonal block is fully masked: overwrite
                    # with -1e30 (dep-ordered after its matmul) so it exps to
                    # zero and the pair stays valid over [offA, 512)
                    nc.vector.memset(ps[:, 512 + offA:512 + offB], -1e30)
                store[(bi, p)] = (ps, offA)

            emit_sp(0, 0)
            pending_rescale = []

            def flush_rescale():
                for po_, rs_, hh_, qc_ in pending_rescale:
                    rrep = rs_pool.tile([128, 512], FP32, tag="rrep")
                    nc.vector.reciprocal_approx_fast(rrep, rs_)
                    nc.vector.tensor_mul(OT[hh_][:, qc_:qc_ + 512], po_, rrep)
                    if hh_ == TPC - 1:
                        filler.extend((qc_ // 512, m) for m in range(NK))
                pending_rescale.clear()

            for bi, (g, hh) in enumerate(blocks):
                qcol0 = g * 512
                nchunks = 4 * g + 4
                npairs = nchunks // 2

                po = psO.tile([128, 512], FP32, tag="psO")
                rs = psO.tile([128, 512], FP32, tag="psO")
                for p in range(npairs):
                    if p + 1 < npairs:
                        emit_sp(bi, p + 1)
                    elif bi + 1 < len(blocks):
                        # last pair: pre-emit the NEXT block's first scores
                        # pair so its exp starts before this block drains
                        emit_sp(bi + 1, 0)
                    ps, offA = store.pop((bi, p))
                    pt = p_pool.tile([128, 1024], FP8, tag="p")
                    ps_v = ps.rearrange("p (two q) -> p two q", two=2)[:, :, offA:512]
                    pt_v = pt.rearrange("p (two q) -> p two q", two=2)[:, :, offA:512]
                    nc.scalar.activation(out=pt_v, in_=ps_v, func=EXP,
                                         scale=SCALE, bias=bias_t)
                    # rowsum over k (ones DoubleRow matmul, psum-accum)
                    nc.tensor.matmul(rs[:, offA:512], ones_t, pt_v,
                                     start=(p == 0), stop=(p == npairs - 1),
                                     perf_mode=DR)
                    # PV accumulate: two mixed bf16-stat x fp8-mov matmuls;
                    # chunk B's zeroed hole contributes nothing over the
                    # union window
                    for h, kj in ((0, 2 * p), (1, 2 * p + 1)):
                        nc.tensor.matmul(
                            po[:, offA:512],
                            V[kj][:, hh * HD:(hh + 1) * HD],
                            pt[:, 512 * h + offA:512 * h + 512],
                            start=(p == 0 and h == 0),
                            stop=(p == npairs - 1 and h == 1),
                        )
                    if p == 0 and pending_rescale:
                        # deferred rescale: every block's recip/mul chain is
                        # emitted after the NEXT block's first pair, so that
                        # pair's mask-subs enter the in-order DVE queue ahead
                        # of it (the chain otherwise stalls the next scores)
                        flush_rescale()
                    # ration filler to span each stretch: 16 units per
                    # released group vs 32 (3,*) / 24 (2,*) / 16 (1,*)
                    # pairs - a drained queue leaves pairs exp-latency
                    # exposed (psS double-buffering caps the exp lead at
                    # one pair, so no slack accumulates)
                    if g == 3:
                        emit_filler(p % 2)
                    elif g == 2:
                        emit_filler(0 if p % 3 == 0 else 1)
                    else:
                        emit_filler(per_pair[g])
                pending_rescale.append((po, rs, hh, qcol0))

            flush_rescale()
            # ========= phase 3: drain remaining outproj units ==========
            while filler:
                emit_outproj_unit(*filler.pop(0))


_PROGRAM = None


def _get_program():
    global _PROGRAM
    if _PROGRAM is None:
        _PROGRAM = build_program()
    return _PROGRAM


def _make_in_maps(x, cos, sin, Wqkv, Wproj):
    maskl = (np.tril(np.ones((128, 128), np.float32), -1) * 1e30).astype(np.float32)
    ident = np.eye(128, dtype=np.float32).astype(NP_BF16)
    ones8 = np.ones((128, 2, 128), dtype=np.float32).astype(NP_FP8)
    cosP = np.ascontiguousarray(
        np.asarray(cos, np.float32).reshape(NT, 128, HALF).transpose(1, 0, 2))
    sinP = np.ascontiguousarray(
        np.asarray(sin, np.float32).reshape(NT, 128, HALF).transpose(1, 0, 2))
    in_maps = []
    for c in range(8):
        b, hg = c // 4, c % 4
        h0 = hg * TPC
        in_maps.append({
            "xT": np.ascontiguousarray(x[b].T).astype(NP_BF16),
            "wqk": np.ascontiguousarray(np.concatenate(
                [Wqkv[:, h0 * HD:(h0 + TPC) * HD],
                 Wqkv[:, D + h0 * HD:D + (h0 + TPC) * HD]],
                axis=1)).astype(NP_BF16),
            "wv": np.ascontiguousarray(
                Wqkv[:, 2 * D + h0 * HD:2 * D + (h0 + TPC) * HD]).astype(NP_BF16),
            "wp": np.ascontiguousarray(
                Wproj[h0 * HD:(h0 + TPC) * HD, :]).astype(NP_BF16),
            "cosP": cosP,
            "sinP": sinP,
            "maskl": maskl,
            "ident": ident,
            "ones8": ones8,
        })
    return in_maps


def _combine(results):
    outs = []
    for b in range(2):
        acc = results[4 * b]["outT"].astype(np.float32)
        for hg in range(1, 4):
            acc = acc + results[4 * b + hg]["outT"].astype(np.float32)
        outs.append(acc.T)
    return np.ascontiguousarray(np.stack(outs))


def kernel(x, cos, sin, Wqkv, Wproj):
    nc = _get_program()
    in_maps = _make_in_maps(np.asarray(x, np.float32), cos, sin,
                            np.asarray(Wqkv, np.float32),
                            np.asarray(Wproj, np.float32))
    res = run_bass_kernel_spmd(nc, in_maps, list(range(8)))
    return _combine(res.results)


def _install_ntff_shim():
    """Provide the antenv.axon_hooks registry this container lacks, wired to
    the ctypes NTFF hook from trn_agent_boot, so trace=True works."""
    import types

    if "antenv.axon_hooks" in sys.modules:
        return
    hook = None
    try:
        from trn_agent_boot.trn_boot import _ntff_profile_via_ctypes
        hook = _ntff_profile_via_ctypes("/opt/axon/libaxon_pjrt.so")
    except Exception as e:
        print("ntff shim unavailable:", e)
    mod = types.ModuleType("antenv.axon_hooks")
    mod._hook = hook
    mod.get_axon_ntff_profile_hook = lambda: mod._hook
    mod.set_axon_ntff_profile_hook = lambda h: setattr(mod, "_hook", h)
    sys.modules["antenv.axon_hooks"] = mod
    # keep artifacts local; the bucket upload path isn't available here
    import concourse.bass_utils as bu
    bu.upload_artifacts = lambda tmpdir: tmpdir


def kernel_profiled(x, cos, sin, Wqkv, Wproj, trace_cores=None, tmpdir=None):
    nc = _get_program()
    _install_ntff_shim()
    in_maps = _make_in_maps(np.asarray(x, np.float32), cos, sin,
                            np.asarray(Wqkv, np.float32),
                            np.asarray(Wproj, np.float32))
    res = run_bass_kernel_spmd(nc, in_maps, list(range(8)), trace=True,
                               trace_cores=trace_cores, tmpdir=tmpdir)
    return _combine(res.results), res


# revision 42
# speedup vs baseline: 1.0407x; 1.0001x over previous
"""Causal self-attention with RoPE on 8 Trainium2 NeuronCores.

Sharding: tensor-parallel over heads (4 groups of 4 heads) x data-parallel
over batch (2), one (batch, head-group) pair per core. Each core computes
its heads' QKV projection, RoPE, causal attention, and a row-slice of the
output projection; the host sums the 4 partial projections per batch.

Precision: all GEMMs run bf16 (fp8 operand noise of 3.6% fails the 2e-2
gate on every path except the probabilities) - except the softmax
probabilities, written once as fp8-e4m3 by the exp. That enables the
rowsum as an fp8 DoubleRow matmul (half cost) and P@V as a mixed
bf16-stationary x fp8-moving matmul (full rate, V stays clean). The
probabilities' quantization noise largely cancels between the P@V
numerator and the rowsum denominator. exp computes exp(s*scale - 2.5);
the bias keeps values under e4m3's 240 max and cancels in normalization.

RoPE exploits that attention is invariant to any per-head permutation of
the q/k feature dim: outputs are stored as [x1*c-x2*s | x1*s+x2*c] halves
(no interleaving), so the DVE chain is 4 batched ops per 128-token tile.

Phase 2 orders head-blocks group-major (g=0,3,2,1) and spreads the output
projection's 64 independent (c4, m) units as PE filler between attention
pairs, so the exp latency never leaves the PE idle.

Hardcoded problem shape: x (2,2048,2048), Wqkv (2048,6144), Wproj
(2048,2048), cos/sin (2048,64), 16 heads, head_dim 128.
"""

import sys

sys.path.insert(0, "/opt/trn_rl_repo")

import numpy as np
import ml_dtypes

import concourse.bass as bass
import concourse.tile as tile
from concourse import bacc, mybir
from concourse.bass_utils import run_bass_kernel_spmd

B, T, D, H = 2, 2048, 2048, 16
HD, HALF = 128, 64
TPC = 4          # heads per core
NT = T // 128    # 16 t-tiles
NK = D // 128    # 16 contraction chunks for the projections
NG = T // 512    # 4 q-groups per head
SCALE = float(1.0 / np.sqrt(HD))
EXP_BIAS = -2.5
FP32 = mybir.dt.float32
BF16 = mybir.dt.bfloat16
FP8 = mybir.dt.float8e4
NP_BF16 = ml_dtypes.bfloat16
NP_FP8 = ml_dtypes.float8_e4m3
DR = mybir.MatmulPerfMode.DoubleRow
EXP = mybir.ActivationFunctionType.Exp


def build_program():
    nc = bacc.Bacc("TRN2", target_bir_lowering=False, debug=False)

    xT = nc.dram_tensor("xT", [D, T], BF16, kind="ExternalInput").ap()
    wqk = nc.dram_tensor("wqk", [D, 2 * TPC * HD], BF16, kind="ExternalInput").ap()
    wv = nc.dram_tensor("wv", [D, TPC * HD], BF16, kind="ExternalInput").ap()
    wp = nc.dram_tensor("wp", [TPC * HD, D], BF16, kind="ExternalInput").ap()
    # cos/sin pre-tiled: [p, t, i] = table[128t + p, i]
    cosP = nc.dram_tensor("cosP", [128, NT, HALF], FP32, kind="ExternalInput").ap()
    sinP = nc.dram_tensor("sinP", [128, NT, HALF], FP32, kind="ExternalInput").ap()
    maskl = nc.dram_tensor("maskl", [128, 128], FP32, kind="ExternalInput").ap()
    ident = nc.dram_tensor("ident", [128, 128], BF16, kind="ExternalInput").ap()
    ones8 = nc.dram_tensor("ones8", [128, 2, 128], FP8, kind="ExternalInput").ap()
    outT = nc.dram_tensor("outT", [D, T], BF16, kind="ExternalOutput").ap()

    with tile.TileContext(nc) as tc:
        _kernel(tc, xT, wqk, wv, wp, cosP, sinP, maskl, ident, ones8, outT)
    nc.compile()
    return nc


def _kernel(tc, xT, wqk, wv, wp, cosP, sinP, maskl, ident, ones8, outT):
    nc = tc.nc
    NQK = 2 * TPC * HD  # 1024 qk output columns
    NV = TPC * HD       # 512 v output columns

    from contextlib import ExitStack

    with ExitStack() as top:
        # ---- persistent pools ----
        consts = top.enter_context(tc.tile_pool(name="consts", bufs=1))
        wp_pool = top.enter_context(tc.tile_pool(name="wp", bufs=TPC))
        qk_pool = top.enter_context(tc.tile_pool(name="qk", bufs=2))
        v_pool = top.enter_context(tc.tile_pool(name="v", bufs=NT))
        o_pool = top.enter_context(tc.tile_pool(name="o", bufs=TPC))

        # phase-2-only constants are DMA'd at phase-2 start so they don't
        # inflate the first projection matmul's DMA semaphore threshold
        l_tile = consts.tile([128, 128], FP32)
        ones_t = consts.tile([128, 2, 128], FP8)
        bias_t = consts.tile([128, 1], FP32)
        id_tile = consts.tile([128, 128], BF16)

        # QT/KT: (128 hd, head, T) single tiles so one DVE copy evicts all
        # four heads' transposes; V: per t-tile (128 t, 4*HD)
        QT = qk_pool.tile([128, TPC, T], BF16, name="QT")
        KT = qk_pool.tile([128, TPC, T], BF16, name="KT")
        V = [v_pool.tile([128, NV], BF16, tag="v", name=f"V{i}")
             for i in range(NT)]
        # attention output transposed: per head, (128 hd, T)
        OT = [o_pool.tile([128, T], BF16, tag="o", name=f"OT{i}")
              for i in range(TPC)]

        # ===== phase 1: fused q/k/v projection + rope + transpose ==========
        # Single pass over x: per 128-row t-tile, accumulate Q,K (one 2-bank
        # psum) and V over the 16 D-chunks, then rope+transpose Q/K and
        # evict V.
        with tc.tile_pool(name="wqk_cache", bufs=NK // 4) as wqk_pool, \
             tc.tile_pool(name="wv_cache", bufs=NK // 4) as wv_pool, \
             tc.tile_pool(name="x_stream", bufs=6) as x_pool, \
             tc.tile_pool(name="rope_tmp", bufs=2) as rope_pool, \
             tc.tile_pool(name="ro_sb", bufs=3) as ro_pool, \
             tc.tile_pool(name="cs", bufs=1) as cs_pool, \
             tc.tile_pool(name="psA", bufs=2, space="PSUM") as psA, \
             tc.tile_pool(name="psV", bufs=2, space="PSUM") as psVp, \
             tc.tile_pool(name="psT", bufs=2, space="PSUM") as psT:

            cos_t = cs_pool.tile([128, NT * HALF], FP32)
            sin_t = cs_pool.tile([128, NT * HALF], FP32)

            # weight-cache tiles are DMA'd lazily inside the t=0 loop so the
            # first matmul's DMA semaphore wait covers only what it needs
            WQK = []
            WV = []

            def emit_transposes(pend):
                # transposes of tile t-1, emitted under tile t's matmuls so
                # the PE never waits on the serial DVE rope chain
                for ro, g, pst, hh in pend:
                    nc.tensor.transpose(
                        pst[:, hh * 128:(hh + 1) * 128], ro[:, g], id_tile)

            pend_tr = []
            pend_ev = []
            WP = []
            XW = [None] * NK
            for t in range(NT):
                psQK = psA.tile([128, 1024], FP32, tag="psA")
                psV = psVp.tile([128, 512], FP32, tag="psV")
                tg0 = (t // 4) * 512   # t-group column base
                tc0 = (t % 4) * 128    # this tile's offset within the group
                for k in range(NK):
                    kj, kk = k // 4, k % 4
                    if t == 0 and kk == 0:
                        # 4-chunk merged weight DMAs: 4x fewer Sync issues
                        # (the ~0.6us per-DMA descriptor cost made t=0
                        # issue-bound) and smaller conservative semaphore
                        # thresholds for the early matmuls. The very first
                        # group is halved, with the second half (and the
                        # transpose identity, first used at t=1) emitted
                        # after the k=0 matmuls so matmul-0's conservative
                        # threshold covers only ~0.75MB of transfers.
                        w = wqk_pool.tile([128, 4, NQK], BF16, tag="wqk")
                        if kj == 0:
                            nc.sync.dma_start(
                                out=w[:, 0:2, :],
                                in_=wqk[0:256, :].rearrange(
                                    "(kk p) c -> p kk c", kk=2))
                        else:
                            nc.sync.dma_start(
                                out=w,
                                in_=wqk[kj * 512:(kj + 1) * 512, :].rearrange(
                                    "(kk p) c -> p kk c", kk=4))
                        WQK.append(w)
                    if t == 0 and k == 1:
                        nc.sync.dma_start(out=id_tile, in_=ident)
                        nc.sync.dma_start(
                            out=WQK[0][:, 2:4, :],
                            in_=wqk[256:512, :].rearrange(
                                "(kk p) c -> p kk c", kk=2))
                        nc.sync.dma_start(
                            out=XW[0][:, 2:4, :],
                            in_=xT[256:512, tg0:tg0 + 512].rearrange(
                                "(kk p) t -> p kk t", kk=2))
                    if t % 4 == 0 and kk == 0:
                        # x streams in merged (4 d-chunk, 512 t) tiles
                        xw = x_pool.tile([128, 4, 512], BF16, tag="x")
                        if t == 0 and kj == 0:
                            nc.sync.dma_start(
                                out=xw[:, 0:2, :],
                                in_=xT[0:256, tg0:tg0 + 512].rearrange(
                                    "(kk p) t -> p kk t", kk=2))
                        else:
                            nc.sync.dma_start(
                                out=xw,
                                in_=xT[kj * 512:(kj + 1) * 512,
                                       tg0:tg0 + 512].rearrange(
                                    "(kk p) t -> p kk t", kk=4))
                        XW[kj] = xw
                        if t == 0 and k == 0:
                            # cos/sin ride the GpSimd DMA queue so they
                            # don't delay the weight stream on Sync
                            nc.gpsimd.dma_start(out=cos_t, in_=cosP)
                            nc.gpsimd.dma_start(out=sin_t, in_=sinP)
                    xt = XW[kj][:, kk, tc0:tc0 + 128]
                    nc.tensor.matmul(psQK[:, 0:512], xt, WQK[kj][:, kk, 0:512],
                                     start=(k == 0), stop=(k == NK - 1))
                    nc.tensor.matmul(psQK[:, 512:1024], xt, WQK[kj][:, kk, 512:1024],
                                     start=(k == 0), stop=(k == NK - 1))
                    if t == 0 and kk == 0:
                        # wv load emitted after the q/k matmuls that don't
                        # need it: keeps it out of their DMA sem threshold
                        w = wv_pool.tile([128, 4, NV], BF16, tag="wv")
                        nc.sync.dma_start(
                            out=w,
                            in_=wv[kj * 512:(kj + 1) * 512, :].rearrange(
                                "(kk p) c -> p kk c", kk=4))
                        WV.append(w)
                    nc.tensor.matmul(psV, xt, WV[kj][:, kk, :],
                                     start=(k == 0), stop=(k == NK - 1))
                    if k in (3, 5, 7, 9) and pend_tr:
                        # prior tile's rope outputs are ready by now; slot its
                        # transposes pairwise between this tile's matmuls
                        emit_transposes(pend_tr[:2])
                        pend_tr = pend_tr[2:]
                    if k == 11 and pend_ev:
                        # evict prior tile's transposes (DVE, packed bf16)
                        for pst, dst, tcol in pend_ev:
                            nc.vector.tensor_copy(
                                out=dst[:, :, tcol:tcol + 128],
                                in_=pst.rearrange("p (h c) -> p h c", h=TPC))
                        pend_ev = []
                # evict V on ACT: DVE is busy with rope in this phase
                nc.scalar.copy(out=V[t], in_=psV)

                # ---- rope, batched over {q,k} x 4 heads (g = qk*4+hh) ----
                # psQK layout: [qk, head, half, i]; out halves stay split
                # ([x1c-x2s | x1s+x2c]) - attention is invariant to the
                # per-head feature permutation, V/outproj are untouched.
                src = psQK.rearrange("p (g half i) -> p g half i",
                                     g=2 * TPC, half=2)
                c_b = cos_t[:, t * HALF:(t + 1) * HALF].unsqueeze(1).unsqueeze(1) \
                    .broadcast_to((128, 2 * TPC, 2, HALF))
                s_b = sin_t[:, t * HALF:(t + 1) * HALF].unsqueeze(1).unsqueeze(1) \
                    .broadcast_to((128, 2 * TPC, 2, HALF))
                t_a = rope_pool.tile([128, 2 * TPC, 2, HALF], BF16, tag="ta")
                t_b = rope_pool.tile([128, 2 * TPC, 2, HALF], BF16, tag="tb")
                nc.vector.tensor_mul(t_a, src, c_b)
                nc.vector.tensor_mul(t_b, src, s_b)
                ro = ro_pool.tile([128, 2 * TPC, 2, HALF], BF16, tag="ro")
                nc.vector.tensor_sub(ro[:, :, 0], t_a[:, :, 0], t_b[:, :, 1])
                nc.vector.tensor_add(ro[:, :, 1], t_b[:, :, 0], t_a[:, :, 1])
                pstQ = psT.tile([128, 512], BF16, tag="psT")
                pstK = psT.tile([128, 512], BF16, tag="psT")
                for hh in range(TPC):
                    pend_tr.append((ro, hh, pstQ, hh))
                for hh in range(TPC):
                    pend_tr.append((ro, TPC + hh, pstK, hh))
                pend_ev = [(pstQ, QT, t * 128), (pstK, KT, t * 128)]
            emit_transposes(pend_tr)
            for pst, dst, tcol in pend_ev:
                nc.vector.tensor_copy(
                    out=dst[:, :, tcol:tcol + 128],
                    in_=pst.rearrange("p (h c) -> p h c", h=TPC))

        # ===== phase 2: causal attention + interleaved output proj =========
        # Score chunk pairs land in one 2-bank psum tile; one exp covers the
        # pair's union window (ragged holes are computed then -1e30'd so
        # they exp to zero), producing fp8 probabilities. P@V runs as two
        # mixed bf16xfp8 matmuls per pair; the rowsum as one fp8 DoubleRow.
        # Output-projection units are spread between pairs as PE filler.
        with tc.tile_pool(name="p_sb", bufs=4) as p_pool, \
             tc.tile_pool(name="rs_sb", bufs=4) as rs_pool, \
             tc.tile_pool(name="out_evict", bufs=6) as out_pool, \
             tc.tile_pool(name="psS", bufs=2, space="PSUM") as psS, \
             tc.tile_pool(name="psO", bufs=4, space="PSUM") as psO:
            # prefetch phase-2 constants + Wproj now (not at kernel start)
            nc.sync.dma_start(out=l_tile, in_=maskl)
            nc.sync.dma_start(out=ones_t, in_=ones8)
            nc.gpsimd.memset(bias_t, EXP_BIAS)
            for hh in range(TPC):
                w = wp_pool.tile([128, D], BF16, tag="wp", name=f"WP{hh}")
                nc.sync.dma_start(out=w, in_=wp[hh * 128:(hh + 1) * 128, :])
                WP.append(w)
            filler = []      # ready outproj (c4, m) units
            n_evict = [0]

            def emit_outproj_unit(c4, m):
                ps = psO.tile([128, 512], FP32, tag="psO")
                for hh2 in range(TPC):
                    nc.tensor.matmul(
                        ps,
                        WP[hh2][:, m * 128:(m + 1) * 128],
                        OT[hh2][:, c4 * 512:(c4 + 1) * 512],
                        start=(hh2 == 0), stop=(hh2 == TPC - 1),
                    )
                ob = out_pool.tile([128, 512], BF16, tag="ob")
                # alternate evict engine: DVE carries most, ACT takes every
                # third (ACT has headroom over the exp stream); the out DMA
                # issue rotates across queues so the drain never serializes
                # on the Sync engine's ~0.6us per-DMA descriptor cost
                if n_evict[0] % 3 == 2:
                    nc.scalar.copy(out=ob, in_=ps)
                else:
                    nc.vector.tensor_copy(out=ob, in_=ps)
                n_evict[0] += 1
                nc.sync.dma_start(
                    out=outT[m * 128:(m + 1) * 128, c4 * 512:(c4 + 1) * 512],
                    in_=ob,
                )

            def emit_filler(k):
                for _ in range(k):
                    if filler:
                        emit_outproj_unit(*filler.pop(0))

            # group-major block order: finishing all heads of a q-group
            # releases that group's 16 outproj units as filler for the rest.
            # g=0 first: the smallest group minimizes the filler-less prefix.
            blocks = [(g, hh) for g in (0, 3, 2, 1) for hh in range(TPC)]
            per_pair = {0: 0, 3: 1, 2: 1, 1: 1}
            store = {}

            def emit_sp(bi, p):
                g, hh = blocks[bi]
                qcol0 = g * 512
                ps = psS.tile([128, 1024], FP32, tag="psS")
                offA = max(0, 2 * p - 4 * g) * 128
                offB = max(0, 2 * p + 1 - 4 * g) * 128
                for h, kj in ((0, 2 * p), (1, 2 * p + 1)):
                    # both chunks computed over the pair's union window
                    # [offA, 512) so one exp / one DoubleRow covers both
                    nc.tensor.matmul(
                        ps[:, 512 * h + offA:512 * h + 512],
                        KT[:, hh, kj * 128:(kj + 1) * 128],
                        QT[:, hh, qcol0 + offA:qcol0 + 512],
                        start=True, stop=True,
                    )
                    # causal mask emitted WITH the scores: resolved well
                    # before the consuming exp
                    sd = kj - 4 * g
                    if 0 <= sd <= 3:
                        dcol = 512 * h + sd * 128
                        nc.vector.tensor_sub(
                            ps[:, dcol:dcol + 128],
                            ps[:, dcol:dcol + 128],
                            l_tile,
                        )
                if offB > offA:
                    # chunk B's pre-diagonal block is fully masked: overwrite
                    # with -1e30 (dep-ordered after its matmul) so it exps to
                    # zero and the pair stays valid over [offA, 512)
                    nc.vector.memset(ps[:, 512 + offA:512 + offB], -1e30)
                store[(bi, p)] = (ps, offA)

            emit_sp(0, 0)
            pending_rescale = []

            def flush_rescale():
                for po_, rs_, hh_, qc_ in pending_rescale:
                    rrep = rs_pool.tile([128, 512], FP32, tag="rrep")
                    nc.vector.reciprocal_approx_fast(rrep, rs_)
                    nc.vector.tensor_mul(OT[hh_][:, qc_:qc_ + 512], po_, rrep)
                    if hh_ == TPC - 1:
                        filler.extend((qc_ // 512, m) for m in range(NK))
                pending_rescale.clear()

            for bi, (g, hh) in enumerate(blocks):
                qcol0 = g * 512
                nchunks = 4 * g + 4
                npairs = nchunks // 2

                po = psO.tile([128, 512], FP32, tag="psO")
                rs = psO.tile([128, 512], FP32, tag="psO")
                for p in range(npairs):
                    if p + 1 < npairs:
                        emit_sp(bi, p + 1)
                    elif bi + 1 < len(blocks):
                        # last pair: pre-emit the NEXT block's first scores
                        # pair so its exp starts before this block drains
                        emit_sp(bi + 1, 0)
                    ps, offA = store.pop((bi, p))
                    pt = p_pool.tile([128, 1024], FP8, tag="p")
                    ps_v = ps.rearrange("p (two q) -> p two q", two=2)[:, :, offA:512]
                    pt_v = pt.rearrange("p (two q) -> p two q", two=2)[:, :, offA:512]
                    nc.scalar.activation(out=pt_v, in_=ps_v, func=EXP,
                                         scale=SCALE, bias=bias_t)
                    # rowsum over k (ones DoubleRow matmul, psum-accum)
                    nc.tensor.matmul(rs[:, offA:512], ones_t, pt_v,
                                     start=(p == 0), stop=(p == npairs - 1),
                                     perf_mode=DR)
                    # PV accumulate: two mixed bf16-stat x fp8-mov matmuls;
                    # chunk B's zeroed hole contributes nothing over the
                    # union window
                    for h, kj in ((0, 2 * p), (1, 2 * p + 1)):
                        nc.tensor.matmul(
                            po[:, offA:512],
                            V[kj][:, hh * HD:(hh + 1) * HD],
                            pt[:, 512 * h + offA:512 * h + 512],
                            start=(p == 0 and h == 0),
                            stop=(p == npairs - 1 and h == 1),
                        )
                    if p == 0 and pending_rescale:
                        # deferred rescale: every block's recip/mul chain is
                        # emitted after the NEXT block's first pair, so that
                        # pair's mask-subs enter the in-order DVE queue ahead
                        # of it (the chain otherwise stalls the next scores)
                        flush_rescale()
                    # ration filler to span each stretch: 16 units per
                    # released group vs 32 (3,*) / 24 (2,*) / 16 (1,*)
                    # pairs - a drained queue leaves pairs exp-latency
                    # exposed (psS double-buffering caps the exp lead at
                    # one pair, so no slack accumulates)
                    if g == 3:
                        emit_filler(p % 2)
                    elif g == 2:
                        emit_filler(0 if p % 3 == 0 else 1)
                    else:
                        emit_filler(per_pair[g])
                pending_rescale.append((po, rs, hh, qcol0))

            flush_rescale()
            # ========= phase 3: drain remaining outproj units ==========
            while filler:
                emit_outproj_unit(*filler.pop(0))


_PROGRAM = None


def _get_program():
    global _PROGRAM
    if _PROGRAM is None:
        _PROGRAM = build_program()
    return _PROGRAM


def _make_in_maps(x, cos, sin, Wqkv, Wproj):
    maskl = (np.tril(np.ones((128, 128), np.float32), -1) * 1e30).astype(np.float32)
    ident = np.eye(128, dtype=np.float32).astype(NP_BF16)
    ones8 = np.ones((128, 2, 128), dtype=np.float32).astype(NP_FP8)
    cosP = np.ascontiguousarray(
        np.asarray(cos, np.float32).reshape(NT, 128, HALF).transpose(1, 0, 2))
    sinP = np.ascontiguousarray(
        np.asarray(sin, np.float32).reshape(NT, 128, HALF).transpose(1, 0, 2))
    in_maps = []
    for c in range(8):
        b, hg = c // 4, c % 4
        h0 = hg * TPC
        in_maps.append({
            "xT": np.ascontiguousarray(x[b].T).astype(NP_BF16),
            "wqk": np.ascontiguousarray(np.concatenate(
                [Wqkv[:, h0 * HD:(h0 + TPC) * HD],
                 Wqkv[:, D + h0 * HD:D + (h0 + TPC) * HD]],
                axis=1)).astype(NP_BF16),
            "wv": np.ascontiguousarray(
                Wqkv[:, 2 * D + h0 * HD:2 * D + (h0 + TPC) * HD]).astype(NP_BF16),
            "wp": np.ascontiguousarray(
                Wproj[h0 * HD:(h0 + TPC) * HD, :]).astype(NP_BF16),
            "cosP": cosP,
            "sinP": sinP,
            "maskl": maskl,
            "ident": ident,
            "ones8": ones8,
        })
    return in_maps


def _combine(results):
    outs = []
    for b in range(2):
        acc = results[4 * b]["outT"].astype(np.float32)
        for hg in range(1, 4):
            acc = acc + results[4 * b + hg]["outT"].astype(np.float32)
        outs.append(acc.T)
    return np.ascontiguousarray(np.stack(outs))


def kernel(x, cos, sin, Wqkv, Wproj):
    nc = _get_program()
    in_maps = _make_in_maps(np.asarray(x, np.float32), cos, sin,
                            np.asarray(Wqkv, np.float32),
                            np.asarray(Wproj, np.float32))
    res = run_bass_kernel_spmd(nc, in_maps, list(range(8)))
    return _combine(res.results)


def _install_ntff_shim():
    """Provide the antenv.axon_hooks registry this container lacks, wired to
    the ctypes NTFF hook from trn_agent_boot, so trace=True works."""
    import types

    if "antenv.axon_hooks" in sys.modules:
        return
    hook = None
    try:
        from trn_agent_boot.trn_boot import _ntff_profile_via_ctypes
        hook = _ntff_profile_via_ctypes("/opt/axon/libaxon_pjrt.so")
    except Exception as e:
        print("ntff shim unavailable:", e)
    mod = types.ModuleType("antenv.axon_hooks")
    mod._hook = hook
    mod.get_axon_ntff_profile_hook = lambda: mod._hook
    mod.set_axon_ntff_profile_hook = lambda h: setattr(mod, "_hook", h)
    sys.modules["antenv.axon_hooks"] = mod
    # keep artifacts local; the bucket upload path isn't available here
    import concourse.bass_utils as bu
    bu.upload_artifacts = lambda tmpdir: tmpdir


def kernel_profiled(x, cos, sin, Wqkv, Wproj, trace_cores=None, tmpdir=None):
    nc = _get_program()
    _install_ntff_shim()
    in_maps = _make_in_maps(np.asarray(x, np.float32), cos, sin,
                            np.asarray(Wqkv, np.float32),
                            np.asarray(Wproj, np.float32))
    res = run_bass_kernel_spmd(nc, in_maps, list(range(8)), trace=True,
                               trace_cores=trace_cores, tmpdir=tmpdir)
    return _combine(res.results), res
